# revision 1
# baseline (speedup 1.0000x reference)
"""CrossAttention Trainium2 kernel (8-core SPMD).

Sharding: core c = (b, g) with b = c // 2 (batch), g = c % 2 (head group of 8).
Each core computes the full attention + partial output projection for its
(batch, 8-head group); the host sums the two partial o-proj results per batch.

Per-core device pipeline (all matmuls fp32r, N=512):
  1. PE-transpose x[b], enc[b] -> xT, eT (C on partitions).
  2. Projections in natural layout: Q,K (T part, 8h x 64d free), V likewise;
     l2-norm (free-dim reduce) + partial rotary applied in natural layout.
  3. PE-transpose Q,K -> qT,kT (head-dims on partitions, T free).
  4. scoresT[k,q] = K @ Q^T accumulated in PSUM with PE-transposed bias tiles;
     exp on ACT; causal masking via memset + triangular-mask multiply;
     AV via lhsT = [V | ones] giving y^T and softmax denominators in one pass.
  5. Normalize y^T by the broadcast reciprocal denominator; o-proj from the
     head-pair-stacked y^T; DMA partial (T, C) result out.
"""

import os
import sys
from contextlib import ExitStack

import numpy as np

if not os.path.isdir(os.path.join(os.path.dirname(os.path.abspath(__file__)), "concourse")):
    for _p in ("/opt/trn_rl_repo",):
        if os.path.isdir(_p) and _p not in sys.path:
            sys.path.insert(0, _p)

import concourse.bass as bass  # noqa: E402
import concourse.tile as tile  # noqa: E402
from concourse import bacc, mybir  # noqa: E402
from concourse.bass_utils import run_bass_kernel_spmd  # noqa: E402

B, T, C = 4, 1024, 1024
H, KV, D = 16, 8, 64
L = 32
HG = 8          # heads per group (= kv heads; local head l uses kv head l)
NG = 2          # head groups
QK_NORM_SCALE = 10.0
DS = float(D) ** -0.5
SCALE_Q = DS * DS / QK_NORM_SCALE   # folded into q's rsqrt(norm) factor

F32 = mybir.dt.float32
F32R = mybir.dt.float32r

NT = T // 128   # 8 T-tiles
NC_ = C // 128  # 8 C-tiles


def r(ap):
    return ap.bitcast(F32R)


def build_program():
    nc = bacc.Bacc(
        "TRN2",
        target_bir_lowering=False,
        debug=False,
        enable_asserts=False,
        num_devices=8,
    )

    def din(name, shape):
        return nc.dram_tensor(name, shape, F32, kind="ExternalInput").ap()

    xb = din("xb", (T, C))
    eb = din("eb", (T, C))
    wq = din("wq", (C, HG * D))
    wk = din("wk", (C, KV * D))
    wv = din("wv", (C, KV * D))
    wo = din("wo", (HG * D, C))
    bias = nc.dram_tensor("bias", (HG, T, T), mybir.dt.bfloat16,
                          kind="ExternalInput").ap()
    cfq = din("cfq", (T, D))
    seq_ = din("seq", (T, L // 2))
    soq = din("soq", (T, L // 2))
    cfk = din("cfk", (T, D))
    sek = din("sek", (T, L // 2))
    sok = din("sok", (T, L // 2))
    cfv = din("cfv", (T, D))
    sev = din("sev", (T, L // 2))
    sov = din("sov", (T, L // 2))
    identf = din("identf", (128, 128))
    tri = din("tri", (128, 128))
    out_d = nc.dram_tensor("out", (T, C), F32, kind="ExternalOutput").ap()

    with tile.TileContext(nc) as tc, ExitStack() as ctx:
        const = ctx.enter_context(tc.tile_pool(name="const", bufs=1))
        persist = ctx.enter_context(tc.tile_pool(name="persist", bufs=1))

        # ---- constants ----
        identr = const.tile([128, 128], F32R, tag="identr")
        nc.sync.dma_start(identr[:], r(identf))
        identb = const.tile([128, 128], mybir.dt.bfloat16, tag="identb")
        nc.vector.tensor_copy(identb[:], identr[:].bitcast(F32))

        natp_ctx = ExitStack()
        natp_outer = natp_ctx.enter_context(tc.tile_pool(name="natp", bufs=2))
        nats = {}

        def load_nat(phase, srcd, half):
            nat = natp_outer.tile([128, 4 * C], F32R, tag="nat",
                                  name=f"nat{phase}{half}")
            nat3 = nat.rearrange("p (tt c) -> p tt c", tt=4)
            nc.sync.dma_start(
                nat3,
                r(srcd[half * 512:(half + 1) * 512, :]
                  .rearrange("(tt p) c -> p tt c", p=128)))
            nats[(phase, half)] = nat3

        load_nat("x", xb, 0)
        load_nat("x", xb, 1)

        # rope constants: (T, n) -> (128, NT, n); loaded later (DMA order)
        rope_sb = {}

        def load_rope_consts():
            for nm, ap_, w in (
                ("cfq", cfq, D), ("seq", seq_, 16), ("soq", soq, 16),
                ("cfk", cfk, D), ("sek", sek, 16), ("sok", sok, 16),
                ("cfv", cfv, D), ("sev", sev, 16), ("sov", sov, 16),
            ):
                t_ = const.tile([128, NT * w], F32, tag=nm, name=nm)
                t3 = t_.rearrange("p (tt d) -> p tt d", tt=NT)
                nc.sync.dma_start(t3, ap_.rearrange("(tt p) d -> p tt d", p=128))
                rope_sb[nm] = t3

        # persistent across attention: wo (loaded later), qT/kT, va
        wo_t = persist.tile([128, 4 * C], F32R, tag="wo", name="wo_t")
        wo_sb = wo_t.rearrange("p (pl c) -> p pl c", pl=4)

        def load_wo_trim():
            nc.sync.dma_start(wo_sb, r(wo.rearrange("(pl p) c -> p pl c", p=128)))
        qT = {(pl, h): persist.tile([128, 512], F32R, tag=f"qT{pl}_{h}",
                                    name=f"qT{pl}_{h}")
              for pl in range(4) for h in range(2)}
        kT = {(pl, h): persist.tile([128, 512], F32R, tag=f"kT{pl}_{h}",
                                    name=f"kT{pl}_{h}")
              for pl in range(4) for h in range(2)}
        va = [persist.tile([128, HG * 65], F32R, tag=f"va{tt}", name=f"va{tt}") for tt in range(NT)]

        def rope_inplace(v3, tt, cf, se, so, smallp):
            """v3: (128, HG, d) SBUF view; applies partial rotary in place."""
            ev = v3[:, :, 0:L:2]
            od = v3[:, :, 1:L:2]
            se_b = rope_sb[se][:, tt].unsqueeze(1).broadcast_to([128, HG, 16])
            so_b = rope_sb[so][:, tt].unsqueeze(1).broadcast_to([128, HG, 16])
            cf_b = rope_sb[cf][:, tt].unsqueeze(1).broadcast_to([128, HG, D])
            tmp_e = smallp.tile([128, HG * 16], F32, tag="tmpe", name="tmpe")
            tmp_o = smallp.tile([128, HG * 16], F32, tag="tmpo", name="tmpo")
            te3 = tmp_e.rearrange("p (h d) -> p h d", h=HG)
            to3 = tmp_o.rearrange("p (h d) -> p h d", h=HG)
            nc.vector.tensor_mul(te3, od, se_b)
            nc.vector.tensor_mul(to3, ev, so_b)
            nc.gpsimd.tensor_mul(v3[:, :, 0:D], v3[:, :, 0:D], cf_b)
            nc.vector.tensor_sub(ev, ev, te3)
            nc.vector.tensor_add(od, od, to3)

        def flush_qn(qns, ttg, tpsum, dstT):
            """PE-transpose 4 ready qn tiles into dstT[pl][:, ttg*512:]."""
            for pl in range(4):
                ps4 = tpsum.tile([128, 512], F32, tag="tps", name="tps")
                for tti in range(4):
                    nc.tensor.matmul(
                        r(ps4[:, tti * 128:(tti + 1) * 128]),
                        qns[tti][:, pl * 128:(pl + 1) * 128],
                        identr[:], is_transpose=True, start=True, stop=True,
                    )
                nc.any.tensor_copy(dstT[(pl, ttg)][:], ps4[:])

        def norm_rope_transpose(ps, tt, which, smallp, sqp, rotp, tpsum, dstT):
            """ps: (128 T, 512) psum of raw projections. Normalizes per head,
            applies rope; returns the qn tile."""
            sq = sqp.tile([128, HG * D], F32, tag="sq", name="sq")
            nc.scalar.square(sq[:], ps[:])
            ss = smallp.tile([128, HG], F32, tag="ss", name="ss")
            nc.vector.tensor_reduce(
                ss[:], sq.rearrange("p (h d) -> p h d", h=HG),
                axis=mybir.AxisListType.X, op=mybir.AluOpType.add,
            )
            inv = smallp.tile([128, HG], F32, tag="inv", name="inv")
            nc.vector.reciprocal(inv[:], ss[:])
            rs = smallp.tile([128, HG], F32, tag="rs", name="rs")
            scl = SCALE_Q * SCALE_Q if which == "q" else 1.0
            nc.scalar.activation(
                rs[:], inv[:], mybir.ActivationFunctionType.Sqrt,
                bias=0.0, scale=scl,
            )
            qn = rotp.tile([128, HG * D], F32R, tag="qn", name="qn")
            d3 = qn.rearrange("p (h d) -> p h d", h=HG)
            nc.vector.tensor_mul(
                d3, ps.rearrange("p (h d) -> p h d", h=HG),
                rs[:].unsqueeze(2).broadcast_to([128, HG, D]),
            )
            if which == "q":
                rope_inplace(d3, tt, "cfq", "seq", "soq", smallp)
            else:
                rope_inplace(d3, tt, "cfk", "sek", "sok", smallp)
            return qn

        # ---- x phase: transpose x -> xT, project Q, -> qT ----
        for phase in ("x", "e"):
            with tc.tile_pool(name="srcT", bufs=1) as srcTp, \
                 tc.tile_pool(name="wp", bufs=1) as wp, \
                 tc.tile_pool(name="projp", bufs=4, space="PSUM") as projp, \
                 tc.tile_pool(name="tpsum", bufs=3, space="PSUM") as tpsum, \
                 tc.tile_pool(name="smallp", bufs=6) as smallp, \
                 tc.tile_pool(name="sqp", bufs=2) as sqp, \
                 tc.tile_pool(name="rotp", bufs=5) as rotp:
                srcT = [srcTp.tile([128, T], F32R, tag=f"sT{cb}", name=f"sT{cb}")
                        for cb in range(NC_)]
                for ttg in range(2):
                    nat3 = nats[(phase, ttg)]
                    for cb in range(NC_):
                        ps4 = tpsum.tile([128, 512], F32, tag="tps",
                                         name="tps")
                        for tti in range(4):
                            nc.tensor.matmul(
                                r(ps4[:, tti * 128:(tti + 1) * 128]),
                                nat3[:, tti, cb * 128:(cb + 1) * 128],
                                identr[:], is_transpose=True,
                                start=True, stop=True,
                            )
                        nc.any.tensor_copy(
                            srcT[cb][:, ttg * 512:(ttg + 1) * 512], ps4[:]
                        )
                if phase == "x":
                    wq_t = wp.tile([128, NC_ * 512], F32R, tag="wq", name="wq_t")
                    wq_sb = wq_t.rearrange("p (cb n) -> p cb n", cb=NC_)
                    nc.sync.dma_start(
                        wq_sb, r(wq.rearrange("(cb p) n -> p cb n", p=128)))
                    load_rope_consts()
                    load_nat("e", eb, 0)
                    load_nat("e", eb, 1)
                    load_wo_trim()
                    qns = []
                    for tt in range(NT):
                        ps = projp.tile([128, 512], F32, tag="proj", name="proj")
                        for cb in range(NC_):
                            nc.tensor.matmul(
                                ps[:], r(srcT[cb][:, tt * 128:(tt + 1) * 128]),
                                r(wq_sb[:, cb]),
                                start=(cb == 0), stop=(cb == NC_ - 1),
                            )
                        qns.append(norm_rope_transpose(ps, tt, "q", smallp,
                                                       sqp, rotp, tpsum, qT))
                        if tt % 4 == 3:
                            flush_qn(qns[-4:], tt // 4, tpsum, qT)
                else:
                    wk_t = wp.tile([128, NC_ * 512], F32R, tag="wk", name="wk_t")
                    wk_sb = wk_t.rearrange("p (cb n) -> p cb n", cb=NC_)
                    nc.sync.dma_start(
                        wk_sb, r(wk.rearrange("(cb p) n -> p cb n", p=128)))
                    wv_t = wp.tile([128, NC_ * 512], F32R, tag="wv", name="wv_t")
                    wv_sb = wv_t.rearrange("p (cb n) -> p cb n", cb=NC_)
                    nc.sync.dma_start(
                        wv_sb, r(wv.rearrange("(cb p) n -> p cb n", p=128)))
                    kns = []
                    for tt in range(NT):
                        ps = projp.tile([128, 512], F32, tag="proj", name="proj")
                        for cb in range(NC_):
                            nc.tensor.matmul(
                                ps[:], r(srcT[cb][:, tt * 128:(tt + 1) * 128]),
                                r(wk_sb[:, cb]),
                                start=(cb == 0), stop=(cb == NC_ - 1),
                            )
                        kns.append(norm_rope_transpose(ps, tt, "k", smallp,
                                                       sqp, rotp, tpsum, kT))
                        if tt % 4 == 3:
                            flush_qn(kns[-4:], tt // 4, tpsum, kT)
                        # V: no norm; pack into 65-stride with ones column
                        psv = projp.tile([128, 512], F32, tag="proj", name="projv")
                        for cb in range(NC_):
                            nc.tensor.matmul(
                                psv[:], r(srcT[cb][:, tt * 128:(tt + 1) * 128]),
                                r(wv_sb[:, cb]),
                                start=(cb == 0), stop=(cb == NC_ - 1),
                            )
                        v3 = va[tt].rearrange("p (h e) -> p h e", h=HG)
                        nc.vector.tensor_copy(
                            v3[:, :, 0:D],
                            psv.rearrange("p (h d) -> p h d", h=HG),
                        )
                        nc.vector.memset(v3[:, :, D:D + 1].bitcast(F32), 1.0)
                        rope_inplace(v3, tt, "cfv", "sev", "sov", smallp)

        natp_ctx.close()

        # ---- attention (qg-outer) + interleaved o-proj ----
        ys = {}
        for pl in range(4):
            for qg in range(2):
                ys[(pl, qg)] = persist.tile([128, 512], F32R,
                                            tag=f"ys{pl}_{qg}",
                                            name=f"ys{pl}_{qg}")

        with tc.tile_pool(name="biasp", bufs=2) as biasp, \
             tc.tile_pool(name="attp", bufs=6) as attp, \
             tc.tile_pool(name="spsum", bufs=4, space="PSUM") as spsum, \
             tc.tile_pool(name="ypsum", bufs=2, space="PSUM") as ypsum, \
             tc.tile_pool(name="opsum", bufs=2, space="PSUM") as opsum, \
             tc.tile_pool(name="outp", bufs=2) as outp, \
             tc.tile_pool(name="smalle", bufs=4) as smalle:

            def oproj(tt):
                ot = outp.tile([128, C], F32, tag="ot", name="ot")
                qg = tt // 4
                for cg in range(2):
                    pso = opsum.tile([128, 512], F32, tag="pso", name="pso")
                    for pl in range(4):
                        nc.tensor.matmul(
                            pso[:],
                            r(ys[(pl, qg)][:, (tt % 4) * 128:(tt % 4 + 1) * 128]),
                            r(wo_sb[:, pl, cg * 512:(cg + 1) * 512]),
                            start=(pl == 0), stop=(pl == 3),
                        )
                    nc.vector.tensor_copy(ot[:, cg * 512:(cg + 1) * 512], pso[:])
                nc.sync.dma_start(out_d[tt * 128:(tt + 1) * 128, :], ot[:])

            for qg in range(2):
                q0 = qg * 512
                nkt = qg * 4 + 4
                qts = range(qg * 4, qg * 4 + 4)
                for lb in range(0, HG, 2):      # head blocks of 2
                    bt = biasp.tile([128, nkt * 2 * 512], mybir.dt.bfloat16,
                                    tag=f"bias{qg}", name=f"bias{qg}_{lb}")
                    bt4 = bt.rearrange("p (h kt q) -> p h kt q", kt=nkt, h=2)
                    for h_ in range(2):
                        nc.sync.dma_start(
                            bt4[:, h_],
                            bias[lb + h_, 0:nkt * 128, q0:q0 + 512]
                            .rearrange("(kt p) q -> p kt q", p=128),
                        )
                    for l4 in range(2):
                        l = lb + l4
                        pl, sub = l // 2, l % 2
                        po = 64 * sub
                        psy = ypsum.tile([65, 512], F32, tag="psy", name="psy")
                        for kt in range(nkt):
                            pss = spsum.tile([128, 512], F32, tag="pss",
                                             name="pss")
                            nc.tensor.matmul(
                                pss[:],
                                r(kT[(pl, kt // 4)][po:po + 64,
                                                    (kt % 4) * 128:(kt % 4 + 1) * 128]),
                                r(qT[(pl, qg)][po:po + 64, :]),
                                start=True, stop=False,
                            )
                            nc.tensor.matmul(
                                pss[:], identb[:], bt4[:, l4, kt, :],
                                start=False, stop=True,
                            )
                            att = attp.tile([128, 512], F32R, tag="att",
                                            name="att")
                            nc.scalar.activation(
                                att[:], pss[:],
                                mybir.ActivationFunctionType.Exp,
                            )
                            nc.tensor.matmul(
                                psy[:],
                                r(va[kt][:, l * 65:(l + 1) * 65]),
                                att[:],
                                start=(kt == 0), stop=(kt == nkt - 1),
                            )
                        rcp = smalle.tile([1, 512], F32, tag="rcp", name="rcp")
                        nc.vector.reciprocal(rcp[:], psy[64:65, :])
                        rb = smalle.tile([64, 512], F32, tag="rb", name="rb")
                        nc.gpsimd.partition_broadcast(rb[:], rcp[:])
                        nc.vector.tensor_mul(
                            ys[(pl, qg)][po:po + 64, :],
                            psy[0:64, :], rb[:],
                        )
                # after all heads of this qg: o-proj for its 4 Tq tiles
                for tt in qts:
                    oproj(tt)

    nc.compile()
    return nc


def host_prep(freqs, q_scale, k_scale):
    """Build rope constant tensors (shared across cores)."""
    c = np.cos(freqs[:, 0::2]).astype(np.float32)   # (T, 16)
    s = np.sin(freqs[:, 0::2]).astype(np.float32)
    consts = {}
    for nm, scale in (("q", q_scale), ("k", k_scale), ("v", np.ones(D, np.float32))):
        scale = np.asarray(scale, np.float32)
        cf = np.empty((T, D), np.float32)
        cf[:, 0:L:2] = c * scale[0:L:2][None, :]
        cf[:, 1:L:2] = c * scale[1:L:2][None, :]
        cf[:, L:] = scale[L:][None, :]
        se = (s * scale[1:L:2][None, :]).astype(np.float32)   # mult q_odd -> even
        so = (s * scale[0:L:2][None, :]).astype(np.float32)   # mult q_even -> odd
        consts[f"cf{nm}"] = np.ascontiguousarray(cf)
        consts[f"se{nm}"] = np.ascontiguousarray(se)
        consts[f"so{nm}"] = np.ascontiguousarray(so)
    consts["identf"] = np.eye(128, dtype=np.float32)
    ii = np.arange(128)
    consts["tri"] = (ii[:, None] <= ii[None, :]).astype(np.float32)
    return consts


_NC_CACHE = {}


def get_nc():
    if "nc" not in _NC_CACHE:
        _NC_CACHE["nc"] = build_program()
    return _NC_CACHE["nc"]


def make_in_maps(x, encoded_data, freqs, attn_bias, Wq, Wk, Wv, Wo,
                 q_scale, k_scale):
    consts = host_prep(np.asarray(freqs, np.float32),
                       np.asarray(q_scale, np.float32),
                       np.asarray(k_scale, np.float32))
    import ml_dtypes
    x = np.asarray(x, np.float32)
    e = np.asarray(encoded_data, np.float32)
    ab = np.asarray(attn_bias, np.float32)
    ii = np.arange(T)
    causal = ii[None, :, None] < ii[None, None, :]   # (1, q, k): k > q masked
    abm = np.where(causal, np.float32(-30.0), ab)    # (H, q, k)
    abT = np.ascontiguousarray(abm.transpose(0, 2, 1)).astype(ml_dtypes.bfloat16)
    Wq = np.asarray(Wq, np.float32)
    Wk = np.ascontiguousarray(np.asarray(Wk, np.float32))
    Wv = np.ascontiguousarray(np.asarray(Wv, np.float32))
    Wo = np.asarray(Wo, np.float32)
    in_maps = []
    for core in range(8):
        b, g = core // 2, core % 2
        m = dict(consts)
        m["xb"] = np.ascontiguousarray(x[b])
        m["eb"] = np.ascontiguousarray(e[b])
        m["wq"] = np.ascontiguousarray(Wq[:, g * 512:(g + 1) * 512])
        m["wk"] = Wk
        m["wv"] = Wv
        m["wo"] = np.ascontiguousarray(Wo[g * 512:(g + 1) * 512, :])
        m["bias"] = np.ascontiguousarray(abT[g * HG:(g + 1) * HG])
        in_maps.append(m)
    return in_maps


def kernel(x, encoded_data, freqs, attn_bias, Wq, Wk, Wv, Wo,
           q_scale, k_scale):
    nc = get_nc()
    in_maps = make_in_maps(x, encoded_data, freqs, attn_bias,
                           Wq, Wk, Wv, Wo, q_scale, k_scale)
    res = run_bass_kernel_spmd(nc, in_maps, core_ids=list(range(8)))
    out = np.empty((B, T, C), np.float32)
    for b in range(B):
        out[b] = res.results[2 * b]["out"] + res.results[2 * b + 1]["out"]
    return out



# revision 2
# speedup vs baseline: 1.0086x; 1.0086x over previous
"""CrossAttention Trainium2 kernel (8-core SPMD), transfer-optimized.

Sharding: core c = (b, g) with b = c // 2 (batch), g = c % 2 (head group of 8).
Each core computes attention + partial o-proj for its (batch, head group);
a pair ReduceScatter sums the two partials on device, each core emitting a
disjoint (512, 1024) half of the batch output in bf16.

Host->device traffic is minimized:
  - x/e halves in bf16, deduplicated across the core pair via AllGather.
  - weights + rope consts in bf16, sharded 4-ways across the head-group's
    cores and AllGathered on device.
  - attn_bias in fp8e4m3, causal-packed (lower 0.75 of T x T), unmasked and
    in natural [q, k] layout: the PE bias-add uses the natural tile as the
    stationary operand with an identity moving operand, which lands bias^T
    into the score PSUM at no extra cycle cost. Causal masking happens on
    device (triangular multiply on diagonal tiles, memset above diagonal).

Per-core device pipeline (all matmuls bf16, N=512):
  1. AllGather x/e pair halves, W blob, bias blob (DRAM bounces).
  2. PE-transpose x, e -> srcT (C on partitions, bf16).
  3. Q/K/V projections (psum fp32); l2-norm + partial rotary; PE-transpose
     Q,K -> qT,kT (head dims on partitions); V packed with ones column.
  4. scoresT[k,q] = K @ Q^T + bias^T (stationary-bias matmuls); exp on ACT;
     causal mask; AV with lhsT = [V | ones] giving y^T and denominators.
  5. Normalize, o-proj, bf16 partial (T, C); pair ReduceScatter -> (512, C).
"""

import os
import sys
from contextlib import ExitStack

import numpy as np

if not os.path.isdir(os.path.join(os.path.dirname(os.path.abspath(__file__)), "concourse")):
    for _p in ("/opt/trn_rl_repo",):
        if os.path.isdir(_p) and _p not in sys.path:
            sys.path.insert(0, _p)

import concourse.bass as bass  # noqa: E402
import concourse.tile as tile  # noqa: E402
from concourse import bacc, mybir  # noqa: E402
from concourse.bass_utils import run_bass_kernel_spmd  # noqa: E402

B, T, C = 4, 1024, 1024
H, KV, D = 16, 8, 64
L = 32
HG = 8          # heads per group (= kv heads; local head l uses kv head l)
QK_NORM_SCALE = 10.0
DS = float(D) ** -0.5
SCALE_Q = DS * DS / QK_NORM_SCALE   # folded into q's rsqrt(norm) factor

F32 = mybir.dt.float32
BF16 = mybir.dt.bfloat16
F8 = mybir.dt.float8e4

NT = T // 128   # 8 T-tiles
NC_ = C // 128  # 8 C-tiles

# ---- W blob layout (bf16 element offsets) ----
_sz_w = C * 512
_off = 0
OFF_WQ = _off; _off += _sz_w
OFF_WK = _off; _off += _sz_w
OFF_WV = _off; _off += _sz_w
OFF_WO = _off; _off += _sz_w
ROPE_SPECS = []  # (name, offset, width)
for _nm, _w in (("cfq", D), ("seq", 16), ("soq", 16),
                ("cfk", D), ("sek", 16), ("sok", 16),
                ("cfv", D), ("sev", 16), ("sov", 16)):
    ROPE_SPECS.append((_nm, _off, _w)); _off += T * _w
OFF_IDENT = _off; _off += 128 * 128
OFF_TRI = _off; _off += 128 * 128
WBLOB = _off                       # 2424832, divisible by 4
assert WBLOB % 4 == 0
WSHARD = WBLOB // 4

# ---- bias blob layout (fp8, per head): block A = [q 0:512, k 0:512],
#      block B = [q 512:1024, k 0:1024]; flat per head = 768K ----
BIAS_A = 512 * 512
BIAS_H = BIAS_A + 512 * 1024       # 786432
BSHARD = HG * BIAS_H // 4          # 1572864


def build_program():
    nc = bacc.Bacc(
        "TRN2",
        target_bir_lowering=False,
        debug=False,
        enable_asserts=False,
        num_devices=8,
    )

    xeh = nc.dram_tensor("xeh", (T, C), BF16, kind="ExternalInput").ap()
    wh = nc.dram_tensor("wh", (WSHARD,), BF16, kind="ExternalInput").ap()
    bh = nc.dram_tensor("bh", (BSHARD,), F8, kind="ExternalInput").ap()
    out_d = nc.dram_tensor("out", (T // 2, C), BF16, kind="ExternalOutput").ap()

    PAIRS = [[0, 1], [2, 3], [4, 5], [6, 7]]
    QUADS = [[0, 2, 4, 6], [1, 3, 5, 7]]

    with tile.TileContext(nc) as tc, ExitStack() as ctx:
        dram = ctx.enter_context(tc.tile_pool(name="dram", bufs=1, space="DRAM"))
        const = ctx.enter_context(tc.tile_pool(name="const", bufs=1))
        persist = ctx.enter_context(tc.tile_pool(name="persist", bufs=1))

        # ---- bounces + collectives (issued early; compute overlaps) ----
        xeh_b = dram.tile([T, C], BF16, tag="xeh_b")
        xe_all = dram.tile([2 * T, C], BF16, tag="xe_all")
        wh_b = dram.tile([WSHARD], BF16, tag="wh_b")
        w_all = dram.tile([WBLOB], BF16, tag="w_all")
        bh_b = dram.tile([BSHARD], F8, tag="bh_b")
        b_all = dram.tile([HG, BIAS_H], F8, tag="b_all")

        nc.gpsimd.dma_start(xeh_b[:], xeh)
        nc.gpsimd.collective_compute(
            "AllGather", mybir.AluOpType.bypass, replica_groups=PAIRS,
            ins=[xeh_b.opt()], outs=[xe_all.opt()],
        )
        nc.gpsimd.dma_start(wh_b[:], wh)
        nc.gpsimd.collective_compute(
            "AllGather", mybir.AluOpType.bypass, replica_groups=QUADS,
            ins=[wh_b.opt()], outs=[w_all.opt()],
        )
        nc.gpsimd.dma_start(bh_b[:], bh)
        nc.gpsimd.collective_compute(
            "AllGather", mybir.AluOpType.bypass, replica_groups=QUADS,
            ins=[bh_b.opt()], outs=[b_all.opt()],
        )

        # gathered xe_all rows: [x_h0; e_h0; x_h1; e_h1] each 512 rows
        def xe_rows(phase, half):
            base = (0 if phase == "x" else 512) + half * 1024
            return xe_all[base:base + 512, :]

        # ---- constants ----
        identb = const.tile([128, 128], BF16, tag="identb")
        nc.sync.dma_start(
            identb[:],
            w_all[OFF_IDENT:OFF_IDENT + 128 * 128].rearrange(
                "(p k) -> p k", p=128))
        trib = const.tile([128, 128], BF16, tag="trib")
        nc.sync.dma_start(
            trib[:],
            w_all[OFF_TRI:OFF_TRI + 128 * 128].rearrange("(p k) -> p k", p=128))

        natp_ctx = ExitStack()
        natp_outer = natp_ctx.enter_context(tc.tile_pool(name="natp", bufs=2))
        nats = {}

        def load_nat(phase, half):
            nat = natp_outer.tile([128, 4 * C], BF16, tag="nat",
                                  name=f"nat{phase}{half}")
            nat3 = nat.rearrange("p (tt c) -> p tt c", tt=4)
            nc.sync.dma_start(
                nat3,
                xe_rows(phase, half).rearrange("(tt p) c -> p tt c", p=128))
            nats[(phase, half)] = nat3

        load_nat("x", 0)
        load_nat("x", 1)

        # rope constants: (T, w) -> (128, NT, w)
        rope_sb = {}

        def load_rope_consts():
            for nm, off, w in ROPE_SPECS:
                t_ = const.tile([128, NT * w], BF16, tag=nm, name=nm)
                t3 = t_.rearrange("p (tt d) -> p tt d", tt=NT)
                nc.sync.dma_start(
                    t3,
                    w_all[off:off + T * w].rearrange(
                        "(tt p d) -> p tt d", tt=NT, p=128))
                rope_sb[nm] = t3

        # persistent across attention: wo (loaded later), qT/kT, va
        wo_t = persist.tile([128, 4 * C], BF16, tag="wo", name="wo_t")
        wo_sb = wo_t.rearrange("p (pl c) -> p pl c", pl=4)

        def load_wo():
            nc.sync.dma_start(
                wo_sb,
                w_all[OFF_WO:OFF_WO + _sz_w].rearrange(
                    "(pl p c) -> p pl c", pl=4, p=128))

        qT = {(pl, h): persist.tile([128, 512], BF16, tag=f"qT{pl}_{h}",
                                    name=f"qT{pl}_{h}")
              for pl in range(4) for h in range(2)}
        kT = {(pl, h): persist.tile([128, 512], BF16, tag=f"kT{pl}_{h}",
                                    name=f"kT{pl}_{h}")
              for pl in range(4) for h in range(2)}
        va = [persist.tile([128, HG * 65], BF16, tag=f"va{tt}", name=f"va{tt}")
              for tt in range(NT)]

        def rope_inplace(v3, tt, cf, se, so, smallp):
            """v3: (128, HG, d) SBUF view (bf16); partial rotary in place."""
            ev = v3[:, :, 0:L:2]
            od = v3[:, :, 1:L:2]
            se_b = rope_sb[se][:, tt].unsqueeze(1).broadcast_to([128, HG, 16])
            so_b = rope_sb[so][:, tt].unsqueeze(1).broadcast_to([128, HG, 16])
            cf_b = rope_sb[cf][:, tt].unsqueeze(1).broadcast_to([128, HG, D])
            tmp_e = smallp.tile([128, HG * 16], F32, tag="tmpe", name="tmpe")
            tmp_o = smallp.tile([128, HG * 16], F32, tag="tmpo", name="tmpo")
            te3 = tmp_e.rearrange("p (h d) -> p h d", h=HG)
            to3 = tmp_o.rearrange("p (h d) -> p h d", h=HG)
            nc.vector.tensor_mul(te3, od, se_b)
            nc.vector.tensor_mul(to3, ev, so_b)
            nc.gpsimd.tensor_mul(v3[:, :, 0:D], v3[:, :, 0:D], cf_b)
            nc.vector.tensor_sub(ev, ev, te3)
            nc.vector.tensor_add(od, od, to3)

        def flush_qn(qns, ttg, tpsum, dstT):
            """PE-transpose 4 ready qn tiles into dstT[pl][:, ttg*512:]."""
            for pl in range(4):
                ps4 = tpsum.tile([128, 512], BF16, tag="tps", name="tps")
                for tti in range(4):
                    nc.tensor.matmul(
                        ps4[:, tti * 128:(tti + 1) * 128],
                        qns[tti][:, pl * 128:(pl + 1) * 128],
                        identb[:], is_transpose=True, start=True, stop=True,
                    )
                nc.any.tensor_copy(dstT[(pl, ttg)][:], ps4[:])

        def norm_rope_transpose(ps, tt, which, smallp, sqp, rotp):
            """ps: (128 T, 512) psum of raw projections. Normalizes per head,
            applies rope; returns the qn tile (bf16)."""
            sq = sqp.tile([128, HG * D], F32, tag="sq", name="sq")
            nc.scalar.square(sq[:], ps[:])
            ss = smallp.tile([128, HG], F32, tag="ss", name="ss")
            nc.vector.tensor_reduce(
                ss[:], sq.rearrange("p (h d) -> p h d", h=HG),
                axis=mybir.AxisListType.X, op=mybir.AluOpType.add,
            )
            inv = smallp.tile([128, HG], F32, tag="inv", name="inv")
            nc.vector.reciprocal(inv[:], ss[:])
            rs = smallp.tile([128, HG], F32, tag="rs", name="rs")
            scl = SCALE_Q * SCALE_Q if which == "q" else 1.0
            nc.scalar.activation(
                rs[:], inv[:], mybir.ActivationFunctionType.Sqrt,
                bias=0.0, scale=scl,
            )
            qn = rotp.tile([128, HG * D], BF16, tag="qn", name="qn")
            d3 = qn.rearrange("p (h d) -> p h d", h=HG)
            nc.vector.tensor_mul(
                d3, ps.rearrange("p (h d) -> p h d", h=HG),
                rs[:].unsqueeze(2).broadcast_to([128, HG, D]),
            )
            if which == "q":
                rope_inplace(d3, tt, "cfq", "seq", "soq", smallp)
            else:
                rope_inplace(d3, tt, "cfk", "sek", "sok", smallp)
            return qn

        # ---- x phase: transpose x -> srcT, project Q, -> qT; e likewise ----
        for phase in ("x", "e"):
            with tc.tile_pool(name="srcT", bufs=1) as srcTp, \
                 tc.tile_pool(name="wp", bufs=1) as wp, \
                 tc.tile_pool(name="projp", bufs=4, space="PSUM") as projp, \
                 tc.tile_pool(name="tpsum", bufs=3, space="PSUM") as tpsum, \
                 tc.tile_pool(name="smallp", bufs=6) as smallp, \
                 tc.tile_pool(name="sqp", bufs=2) as sqp, \
                 tc.tile_pool(name="rotp", bufs=5) as rotp:
                srcT = [srcTp.tile([128, T], BF16, tag=f"sT{cb}", name=f"sT{cb}")
                        for cb in range(NC_)]
                for ttg in range(2):
                    nat3 = nats[(phase, ttg)]
                    for cb in range(NC_):
                        ps4 = tpsum.tile([128, 512], BF16, tag="tps",
                                         name="tps")
                        for tti in range(4):
                            nc.tensor.matmul(
                                ps4[:, tti * 128:(tti + 1) * 128],
                                nat3[:, tti, cb * 128:(cb + 1) * 128],
                                identb[:], is_transpose=True,
                                start=True, stop=True,
                            )
                        nc.any.tensor_copy(
                            srcT[cb][:, ttg * 512:(ttg + 1) * 512], ps4[:]
                        )
                if phase == "x":
                    wq_t = wp.tile([128, NC_ * 512], BF16, tag="wq", name="wq_t")
                    wq_sb = wq_t.rearrange("p (cb n) -> p cb n", cb=NC_)
                    nc.sync.dma_start(
                        wq_sb,
                        w_all[OFF_WQ:OFF_WQ + _sz_w].rearrange(
                            "(cb p n) -> p cb n", cb=NC_, p=128))
                    load_rope_consts()
                    load_nat("e", 0)
                    load_nat("e", 1)
                    load_wo()
                    qns = []
                    for tt in range(NT):
                        ps = projp.tile([128, 512], F32, tag="proj", name="proj")
                        for cb in range(NC_):
                            nc.tensor.matmul(
                                ps[:], srcT[cb][:, tt * 128:(tt + 1) * 128],
                                wq_sb[:, cb],
                                start=(cb == 0), stop=(cb == NC_ - 1),
                            )
                        qns.append(norm_rope_transpose(ps, tt, "q", smallp,
                                                       sqp, rotp))
                        if tt % 4 == 3:
                            flush_qn(qns[-4:], tt // 4, tpsum, qT)
                else:
                    wk_t = wp.tile([128, NC_ * 512], BF16, tag="wk", name="wk_t")
                    wk_sb = wk_t.rearrange("p (cb n) -> p cb n", cb=NC_)
                    nc.sync.dma_start(
                        wk_sb,
                        w_all[OFF_WK:OFF_WK + _sz_w].rearrange(
                            "(cb p n) -> p cb n", cb=NC_, p=128))
                    wv_t = wp.tile([128, NC_ * 512], BF16, tag="wv", name="wv_t")
                    wv_sb = wv_t.rearrange("p (cb n) -> p cb n", cb=NC_)
                    nc.sync.dma_start(
                        wv_sb,
                        w_all[OFF_WV:OFF_WV + _sz_w].rearrange(
                            "(cb p n) -> p cb n", cb=NC_, p=128))
                    kns = []
                    for tt in range(NT):
                        ps = projp.tile([128, 512], F32, tag="proj", name="proj")
                        for cb in range(NC_):
                            nc.tensor.matmul(
                                ps[:], srcT[cb][:, tt * 128:(tt + 1) * 128],
                                wk_sb[:, cb],
                                start=(cb == 0), stop=(cb == NC_ - 1),
                            )
                        kns.append(norm_rope_transpose(ps, tt, "k", smallp,
                                                       sqp, rotp))
                        if tt % 4 == 3:
                            flush_qn(kns[-4:], tt // 4, tpsum, kT)
                        # V: no norm; pack into 65-stride with ones column
                        psv = projp.tile([128, 512], F32, tag="proj", name="projv")
                        for cb in range(NC_):
                            nc.tensor.matmul(
                                psv[:], srcT[cb][:, tt * 128:(tt + 1) * 128],
                                wv_sb[:, cb],
                                start=(cb == 0), stop=(cb == NC_ - 1),
                            )
                        v3 = va[tt].rearrange("p (h e) -> p h e", h=HG)
                        nc.vector.tensor_copy(
                            v3[:, :, 0:D],
                            psv.rearrange("p (h d) -> p h d", h=HG),
                        )
                        nc.vector.memset(v3[:, :, D:D + 1], 1.0)
                        rope_inplace(v3, tt, "cfv", "sev", "sov", smallp)

        natp_ctx.close()

        # ---- attention (qg-outer) + interleaved o-proj ----
        obuf = dram.tile([T, C], BF16, tag="obuf")
        ored = dram.tile([T // 2, C], BF16, tag="ored")

        ys = {}
        for pl in range(4):
            for qg in range(2):
                ys[(pl, qg)] = persist.tile([128, 512], BF16,
                                            tag=f"ys{pl}_{qg}",
                                            name=f"ys{pl}_{qg}")

        with tc.tile_pool(name="biasp", bufs=2) as biasp, \
             tc.tile_pool(name="attp", bufs=6) as attp, \
             tc.tile_pool(name="spsum", bufs=4, space="PSUM") as spsum, \
             tc.tile_pool(name="ypsum", bufs=2, space="PSUM") as ypsum, \
             tc.tile_pool(name="opsum", bufs=2, space="PSUM") as opsum, \
             tc.tile_pool(name="outp", bufs=2) as outp, \
             tc.tile_pool(name="smalle", bufs=4) as smalle:

            def oproj(tt):
                ot = outp.tile([128, C], BF16, tag="ot", name="ot")
                qg = tt // 4
                for cg in range(2):
                    pso = opsum.tile([128, 512], F32, tag="pso", name="pso")
                    for pl in range(4):
                        nc.tensor.matmul(
                            pso[:],
                            ys[(pl, qg)][:, (tt % 4) * 128:(tt % 4 + 1) * 128],
                            wo_sb[:, pl, cg * 512:(cg + 1) * 512],
                            start=(pl == 0), stop=(pl == 3),
                        )
                    nc.vector.tensor_copy(ot[:, cg * 512:(cg + 1) * 512], pso[:])
                nc.sync.dma_start(obuf[tt * 128:(tt + 1) * 128, :], ot[:])

            for qg in range(2):
                nkt = qg * 4 + 4
                qts = range(qg * 4, qg * 4 + 4)
                # natural-layout bias region for this qg: per head,
                # (4*128 q, nkt*128 k) as [p, qi, k]
                boff, bk = (0, 512) if qg == 0 else (BIAS_A, 1024)
                for lb in range(0, HG, 2):      # head blocks of 2
                    bt = biasp.tile([128, 2 * 4 * bk], F8,
                                    tag=f"bias{qg}", name=f"bias{qg}_{lb}")
                    bt4 = bt.rearrange("p (h qi k) -> p h qi k", h=2, qi=4)
                    for h_ in range(2):
                        nc.sync.dma_start(
                            bt4[:, h_],
                            b_all[lb + h_, boff:boff + 4 * 128 * bk]
                            .rearrange("(qi p k) -> p qi k", qi=4, p=128),
                        )
                    for l4 in range(2):
                        l = lb + l4
                        pl, sub = l // 2, l % 2
                        po = 64 * sub
                        psy = ypsum.tile([65, 512], F32, tag="psy", name="psy")
                        for kt in range(nkt):
                            pss = spsum.tile([128, 512], F32, tag="pss",
                                             name="pss")
                            nc.tensor.matmul(
                                pss[:],
                                kT[(pl, kt // 4)][po:po + 64,
                                                  (kt % 4) * 128:(kt % 4 + 1) * 128],
                                qT[(pl, qg)][po:po + 64, :],
                                start=True, stop=False,
                            )
                            # bias^T add: natural [q,k] tile as stationary,
                            # identity moving -> psum[k, q] slices
                            for qi in range(4):
                                nc.tensor.matmul(
                                    pss[:, qi * 128:(qi + 1) * 128],
                                    bt4[:, l4, qi, kt * 128:(kt + 1) * 128],
                                    identb[:],
                                    start=False, stop=(qi == 3),
                                )
                            att = attp.tile([128, 512], BF16, tag="att",
                                            name="att")
                            nc.scalar.activation(
                                att[:], pss[:],
                                mybir.ActivationFunctionType.Exp,
                            )
                            # causal mask: q-block qg*4+qi vs k-block kt
                            for qi in range(4):
                                qb = qg * 4 + qi
                                asl = att[:, qi * 128:(qi + 1) * 128]
                                if kt == qb:
                                    nc.vector.tensor_mul(asl, asl, trib[:])
                                elif kt > qb:
                                    nc.vector.memset(asl, 0.0)
                            nc.tensor.matmul(
                                psy[:],
                                va[kt][:, l * 65:(l + 1) * 65],
                                att[:],
                                start=(kt == 0), stop=(kt == nkt - 1),
                            )
                        rcp = smalle.tile([1, 512], F32, tag="rcp", name="rcp")
                        nc.vector.reciprocal(rcp[:], psy[64:65, :])
                        rb = smalle.tile([64, 512], F32, tag="rb", name="rb")
                        nc.gpsimd.partition_broadcast(rb[:], rcp[:])
                        nc.vector.tensor_mul(
                            ys[(pl, qg)][po:po + 64, :],
                            psy[0:64, :], rb[:],
                        )
                # after all heads of this qg: o-proj for its 4 Tq tiles
                for tt in qts:
                    oproj(tt)

        # ---- pair ReduceScatter of partial outputs; emit half ----
        nc.gpsimd.collective_compute(
            "ReduceScatter", mybir.AluOpType.add, replica_groups=PAIRS,
            ins=[obuf.opt()], outs=[ored.opt()],
        )
        nc.sync.dma_start(out_d, ored[:])

    nc.compile()
    return nc


def host_prep_rope(freqs, q_scale, k_scale):
    """Build rope constant arrays (fp32; cast to bf16 at blob pack)."""
    c = np.cos(freqs[:, 0::2]).astype(np.float32)   # (T, 16)
    s = np.sin(freqs[:, 0::2]).astype(np.float32)
    consts = {}
    for nm, scale in (("q", q_scale), ("k", k_scale),
                      ("v", np.ones(D, np.float32))):
        scale = np.asarray(scale, np.float32)
        cf = np.empty((T, D), np.float32)
        cf[:, 0:L:2] = c * scale[0:L:2][None, :]
        cf[:, 1:L:2] = c * scale[1:L:2][None, :]
        cf[:, L:] = scale[L:][None, :]
        se = (s * scale[1:L:2][None, :]).astype(np.float32)   # mult odd -> even
        so = (s * scale[0:L:2][None, :]).astype(np.float32)   # mult even -> odd
        consts[f"cf{nm}"] = cf
        consts[f"se{nm}"] = se
        consts[f"so{nm}"] = so
    return consts


_NC_CACHE = {}


def get_nc():
    if "nc" not in _NC_CACHE:
        _NC_CACHE["nc"] = build_program()
    return _NC_CACHE["nc"]


def make_in_maps(x, encoded_data, freqs, attn_bias, Wq, Wk, Wv, Wo,
                 q_scale, k_scale):
    import ml_dtypes
    BF = ml_dtypes.bfloat16
    F8NP = ml_dtypes.float8_e4m3

    x = np.asarray(x, np.float32)
    e = np.asarray(encoded_data, np.float32)
    ab = np.asarray(attn_bias, np.float32)
    Wq = np.asarray(Wq, np.float32)
    Wk = np.asarray(Wk, np.float32)
    Wv = np.asarray(Wv, np.float32)
    Wo = np.asarray(Wo, np.float32)
    rope = host_prep_rope(np.asarray(freqs, np.float32),
                          np.asarray(q_scale, np.float32),
                          np.asarray(k_scale, np.float32))
    ii = np.arange(128)

    # W blob per group (bf16)
    wshards = {}
    for g in range(2):
        blob = np.empty((WBLOB,), BF)
        blob[OFF_WQ:OFF_WQ + _sz_w] = \
            Wq[:, g * 512:(g + 1) * 512].astype(BF).ravel()
        blob[OFF_WK:OFF_WK + _sz_w] = Wk.astype(BF).ravel()
        blob[OFF_WV:OFF_WV + _sz_w] = Wv.astype(BF).ravel()
        blob[OFF_WO:OFF_WO + _sz_w] = \
            Wo[g * 512:(g + 1) * 512, :].astype(BF).ravel()
        for nm, off, w in ROPE_SPECS:
            blob[off:off + T * w] = rope[nm].astype(BF).ravel()
        blob[OFF_IDENT:OFF_IDENT + 128 * 128] = \
            np.eye(128, dtype=np.float32).astype(BF).ravel()
        blob[OFF_TRI:OFF_TRI + 128 * 128] = \
            (ii[:, None] <= ii[None, :]).astype(BF).ravel()
        wshards[g] = blob.reshape(4, WSHARD)

    # bias blob per group (fp8, causal-packed, natural [q, k], unmasked)
    bshards = {}
    for g in range(2):
        bg = ab[g * HG:(g + 1) * HG]
        pack = np.empty((HG, BIAS_H), F8NP)
        pack[:, :BIAS_A] = bg[:, 0:512, 0:512].reshape(HG, -1).astype(F8NP)
        pack[:, BIAS_A:] = bg[:, 512:1024, :].reshape(HG, -1).astype(F8NP)
        bshards[g] = pack.reshape(4, BSHARD)

    in_maps = []
    for core in range(8):
        b, g = core // 2, core % 2
        xeh = np.empty((T, C), BF)
        xeh[0:512] = x[b, g * 512:(g + 1) * 512].astype(BF)
        xeh[512:1024] = e[b, g * 512:(g + 1) * 512].astype(BF)
        in_maps.append({
            "xeh": xeh,
            "wh": wshards[g][b],
            "bh": bshards[g][b],
        })
    return in_maps


def kernel(x, encoded_data, freqs, attn_bias, Wq, Wk, Wv, Wo,
           q_scale, k_scale):
    nc = get_nc()
    in_maps = make_in_maps(x, encoded_data, freqs, attn_bias,
                           Wq, Wk, Wv, Wo, q_scale, k_scale)
    res = run_bass_kernel_spmd(nc, in_maps, core_ids=list(range(8)))
    out = np.empty((B, T, C), np.float32)
    for b in range(B):
        out[b, 0:512] = res.results[2 * b]["out"].astype(np.float32)
        out[b, 512:1024] = res.results[2 * b + 1]["out"].astype(np.float32)
    return out
